# revision 27
# baseline (speedup 1.0000x reference)
"""MemEffEquivariantAttention TRN2 Bass kernel (v4).

Sharding: 8 cores = 4 batches x 2 query-token halves (fully data-parallel,
no collectives). Each core computes, for its (batch, 256-token half):
scores -> +bias(masked) -> exp (no max; range-safe) -> u = e*law/Z ->
attn = u @ vf -> equivariant LN -> out_proj.

v4 design:
  - q/k bf16, packed in one qkT tensor (1 DMA trigger per head)
  - bias bf16, identity-matmul lands it in PSUM under the scores
  - Z-normalization DEFERRED off the critical path: u0 = e * (law*2^-32)
    (one 2x-mode bf16 tensor_tensor); the per-head attn output [96,256]
    is normalized instead, via rz broadcast built from a tiny PE
    transpose of z + a 1-partition ones(2^32)-matmul
  - attn outputs land partition-native in X2[96,(h),256]; out_proj runs
    as 96 matmuls with 32-wide contraction against a per-p replicated
    WT (heads 0-7 mid-pipeline, heads 8-15 in the tail) -- no X-stash
    DMA triggers at all
  - per-head pipeline (iteration h): dma(h+2) | scores(h) | exp(h) |
    u0(h) | gather(h) | ztr(h-1) | attn(h-2) | recip+bcast(h-1) |
    atnorm+sq(h-3) | out_proj passA block
"""
import sys
sys.path.insert(0, "/opt/trn_rl_repo")

import numpy as np
import ml_dtypes

import concourse.bacc as bacc
import concourse.tile as tile
from concourse import mybir
from concourse.bass_utils import run_bass_kernel_spmd

F32 = mybir.dt.float32
BF16 = mybir.dt.bfloat16
I16 = mybir.dt.int16
AF = mybir.ActivationFunctionType
ALU = mybir.AluOpType

B, T, P, HID = 4, 512, 3, 512
HD, H = 32, 16
EXP, S = 512, 1024
TQ = 256            # query tokens per core
EPS = 1e-3
CUTOFF = 1e-5
D = P * HD          # 96, per-head feature dim
CS = 2.0 ** -32     # u pre-scale; folded back via ones(2^32) bcast
LAG = 3             # attn(h) emitted at iteration h+LAG

_prog_cache = {}


def _wrap_idx(idx):
    # gpsimd wrapped layout, replicated to all 8 gpsimd cores:
    # idxs[p, s] = idx[s*16 + (p % 16)]
    n = len(idx)
    w = idx.reshape(n // 16, 16).T.astype(np.int16)
    return np.ascontiguousarray(np.tile(w, (8, 1)))


def _build_program():
    nc = bacc.Bacc("TRN2", target_bir_lowering=False, debug=False)

    qkT_d = nc.dram_tensor("qkT", [H, D, S + TQ], BF16, kind="ExternalInput").ap()
    vpk_d = nc.dram_tensor("vpk", [T, H * D], BF16, kind="ExternalInput").ap()
    bias_d = nc.dram_tensor("bias", [H, 2, 128, S], BF16, kind="ExternalInput").ap()
    law_d = nc.dram_tensor("law", [2, 128, S], BF16, kind="ExternalInput").ap()
    WT_d = nc.dram_tensor("WT", [HID, HID], BF16, kind="ExternalInput").ap()
    idv_d = nc.dram_tensor("idv", [128, 32], I16, kind="ExternalInput").ap()
    idt_d = nc.dram_tensor("idt", [2, 128, 8], I16, kind="ExternalInput").ap()
    onesc_d = nc.dram_tensor("onesc", [2, D], BF16, kind="ExternalInput").ap()
    ones_d = nc.dram_tensor("ones96", [D, 1], F32, kind="ExternalInput").ap()
    eye_d = nc.dram_tensor("eye128", [128, 128], BF16, kind="ExternalInput").ap()
    eyef_d = nc.dram_tensor("eyef128", [128, 128], F32, kind="ExternalInput").ap()
    out_d = nc.dram_tensor("out", [TQ, P, HID], F32, kind="ExternalOutput").ap()


    with tile.TileContext(nc) as tc:
        with tc.tile_pool(name="const", bufs=1) as cp, \
             tc.tile_pool(name="work", bufs=3) as wp, \
             tc.tile_pool(name="kq", bufs=3) as kq, \
             tc.tile_pool(name="ug", bufs=6) as ug, \
             tc.tile_pool(name="uT", bufs=6) as up, \
             tc.tile_pool(name="ep", bufs=3) as ep, \
             tc.tile_pool(name="biasp", bufs=3) as bp, \
             tc.tile_pool(name="psw", bufs=2, space="PSUM") as psw, \
             tc.tile_pool(name="psa", bufs=2, space="PSUM") as psa, \
             tc.tile_pool(name="psr", bufs=2, space="PSUM") as psr:

            # ---- constants / preload ----
            v_t = cp.tile([128, 4, H * D], BF16, tag="v")
            vg_t = cp.tile([128, 4, H * D], BF16, tag="vg")
            law_t = cp.tile([128, 2, S], BF16, tag="law")
            WT_t = cp.tile([128, 4, HID], BF16, tag="WT")
            idv_t = cp.tile([128, 32], I16, tag="idv")
            idt_t = cp.tile([128, 2, 8], I16, tag="idt")
            onesc_t = cp.tile([2, D], BF16, tag="onesc")
            ones_t = cp.tile([D, 1], F32, tag="ones")
            eye_t = cp.tile([128, 128], BF16, tag="eye")
            eyef_t = cp.tile([128, 128], F32, tag="eyef")
            X_t = cp.tile([128, P, 4, TQ], BF16, tag="X")
            eps_t = cp.tile([128, 1], F32, tag="eps")
            sqacc_t = cp.tile([D, TQ], F32, tag="sqacc")
            nc.vector.memset(eps_t[:], EPS)

            nc.sync.dma_start(out=eye_t[:], in_=eye_d)
            nc.sync.dma_start(out=eyef_t[:], in_=eyef_d)
            nc.sync.dma_start(out=idt_t[:], in_=idt_d.rearrange("r p c -> p r c"))
            nc.sync.dma_start(out=idv_t[:], in_=idv_d)
            nc.sync.dma_start(out=law_t[:], in_=law_d.rearrange("r p s -> p r s"))
            nc.sync.dma_start(out=onesc_t[:], in_=onesc_d)

            def emit_deferred_preload():
                # needed from the first attn onwards; issued after the
                # first scores so they don't delay the first matmuls
                nc.gpsimd.dma_start(out=v_t[:],
                                    in_=vpk_d.rearrange("(c p) d -> p c d", p=128))
                nc.gpsimd.dma_gather(vg_t[:], vpk_d, idv_t[:],
                                     num_idxs=EXP, num_idxs_reg=EXP,
                                     elem_size=H * D)
                nc.gpsimd.dma_start(out=WT_t[:],
                                    in_=WT_d.rearrange("(c p) o -> p c o", p=128))
                nc.gpsimd.dma_start(out=ones_t[:], in_=ones_d)

            # per-head state carried between pipeline stages
            kq_t = {}
            bias_t = {}
            uT_tiles = {}
            at_tiles = {}
            at_tiles_sb = {}
            z_tiles = {}
            zrow_ps = {}
            rzrow_t = {}
            rzb_sb = {}

            def emit_dma(h):
                kq_t[h] = kq.tile([D, S + TQ], BF16, tag="kq", name=f"kq_{h}")
                bias_t[h] = bp.tile([128, 2, S], BF16, tag="bias",
                                    name=f"bias_{h}")
                nc.sync.dma_start(out=kq_t[h][:], in_=qkT_d[h])
                nc.sync.dma_start(out=bias_t[h][:],
                                  in_=bias_d[h].rearrange("r p s -> p r s"))

            def emit_scores(h):
                # PE: bias (identity) + scores into PSUM. tt-major order so
                # exp(tt0) overlaps the tt1 matmuls and the psw bank for
                # tt0 frees one exp earlier (breaks scores<->exp ping-pong)
                w_tiles = [psw.tile([128, S], F32, tag="w",
                                    name=f"w_{h}_{tt}") for tt in range(2)]
                for tt in range(2):
                    for half in range(2):
                        hs = slice(half * 512, (half + 1) * 512)
                        nc.tensor.matmul(w_tiles[tt][:, hs], eye_t[:],
                                         bias_t[h][:, tt, hs],
                                         start=True, stop=False,
                                         skip_group_check=True)
                    for half in range(2):
                        hs = slice(half * 512, (half + 1) * 512)
                        nc.tensor.matmul(w_tiles[tt][:, hs],
                                         kq_t[h][:, S + tt * 128:S + (tt + 1) * 128],
                                         kq_t[h][:, hs],
                                         start=False, stop=True,
                                         skip_group_check=True)
                return w_tiles

            def emit_exp_u(h, w_tiles):
                # per-tt chain: exp -> u0 -> rz stash -> HALF-gather, so the
                # tt0 gather DMA flies while tt1 is still in exp/DVE
                e_t = ep.tile([128, 2, S], BF16, tag="e", name=f"e_{h}")
                z_t = wp.tile([128, 2], F32, tag="z", name=f"z_{h}")
                rz_t = wp.tile([128, 2], F32, tag="z", name=f"rz_{h}")
                u1_t = ug.tile([128, 2, S + 128], BF16, tag="u1",
                               name=f"u1_{h}")

                uT_t = up.tile([128, 2, 9, 128], BF16, tag="uT",
                               name=f"uT_{h}")
                for tt in range(2):
                    nc.scalar.activation(e_t[:, tt, :], w_tiles[tt][:], AF.Exp,
                                         accum_out=z_t[:, tt:tt + 1])
                    nc.vector.tensor_mul(u1_t[:, tt, 0:S], e_t[:, tt, :],
                                         law_t[:, tt, :])
                    nc.vector.reciprocal(rz_t[:, tt:tt + 1],
                                         z_t[:, tt:tt + 1])
                    nc.vector.tensor_copy(u1_t[:, tt, S:S + 1],
                                          rz_t[:, tt:tt + 1])
                    nc.gpsimd.dma_gather(uT_t[:, tt, :, :],
                                         u1_t[:], idt_t[:, tt, :],
                                         num_idxs=128, num_idxs_reg=128,
                                         elem_size=S + 128, transpose=True,
                                         sbuf_tokens_per_rank=128,
                                         sbuf_free_dim_per_rank=2 * (S + 128))
                uT_tiles[h] = uT_t

            def emit_bcast(h):
                # rzb[0:96, t] = 2^32 * rz[t] broadcast down 96 partitions
                # (rz row came in via the gather: uT[0, 8, :]), then copied
                # to SBUF so the PSUM bank frees this iteration
                zr = psr.tile([128, HID], F32, tag="rzb", name=f"zr_{h}")
                for tt in range(2):
                    nc.tensor.matmul(zr[0:D, tt * 128:(tt + 1) * 128],
                                     onesc_t[0:1, :],
                                     uT_tiles[h][0:1, tt, 8, :],
                                     start=True, stop=True,
                                     skip_group_check=True)
                rzc = wp.tile([D, TQ], F32, tag="rzc", name=f"rzc_{h}")
                nc.vector.tensor_copy(rzc[:], zr[0:D, 0:256])
                rzb_sb[h] = rzc

            def emit_attn(h):
                uT_t = uT_tiles[h]
                at_ps = psa.tile([D, TQ], F32, tag="attn", name=f"at_{h}")
                for tt in range(2):
                    ts = slice(tt * 128, (tt + 1) * 128)
                    for sc in range(8):
                        vsrc = v_t if sc < 4 else vg_t
                        nc.tensor.matmul(at_ps[:, ts],
                                         vsrc[:, sc % 4, h * D:(h + 1) * D],
                                         uT_t[:, tt, sc, :],
                                         start=(sc == 0), stop=(sc == 7))
                at_tiles[h] = at_ps

            def emit_atnorm(h):
                # at_norm = at_ps * rzb -> at_sb; sq accumulate
                at_ps = at_tiles.pop(h)
                uT_tiles.pop(h)
                rzc = rzb_sb.pop(h)
                at_sb = wp.tile([D, TQ], BF16, tag="atsb", name=f"atsb_{h}")
                nc.vector.tensor_mul(at_sb[:], at_ps[:], rzc[:])
                at_tiles_sb[h] = at_sb
                if h == 0:
                    nc.vector.tensor_mul(sqacc_t[:], at_sb[:], at_sb[:])
                else:
                    sq_t = wp.tile([D, TQ], BF16, tag="sq")
                    nc.vector.tensor_mul(sq_t[:], at_sb[:], at_sb[:])
                    nc.vector.tensor_add(sqacc_t[:], sqacc_t[:], sq_t[:])

            def emit_stash(j):
                # X[(j%4)*32+hd, p, j//4, t] <- at_sb[p*32+hd, t]; deferred
                # one head so the triggers' waits are already satisfied
                at_sb = at_tiles_sb.pop(j)
                for p in range(P):
                    eng = nc.scalar if p == 2 else nc.sync
                    eng.dma_start(
                        out=X_t[(j % 4) * 32:(j % 4 + 1) * 32, p, j // 4, :],
                        in_=at_sb[p * 32:(p + 1) * 32, :])

            # ---- main loop ----
            emit_dma(0)
            emit_dma(1)
            for h in range(H):
                if h + 2 < H:
                    emit_dma(h + 2)
                w_tiles = emit_scores(h)
                if h == 0:
                    emit_deferred_preload()
                emit_exp_u(h, w_tiles)
                if h >= LAG:
                    emit_attn(h - LAG)
                if h >= 2:
                    emit_bcast(h - 2)
                if h >= LAG + 1:
                    emit_atnorm(h - LAG - 1)
                if h >= LAG + 2:
                    emit_stash(h - LAG - 2)

            # drain
            emit_bcast(H - 2)
            emit_bcast(H - 1)
            for h in range(H - LAG, H):
                emit_attn(h)
            for h in range(H - LAG - 1, H):
                emit_atnorm(h)
            for h in range(H - LAG - 2, H):
                emit_stash(h)

            # ---- inv = 1/sqrt(mean+eps), out_proj pass B, store ----
            ss_ps = psr.tile([128, HID], F32, tag="rzb", name="ss")
            for tb in range(2):
                nc.tensor.matmul(ss_ps[:, tb:tb + 1],
                                 sqacc_t[:, tb * 128:(tb + 1) * 128],
                                 ones_t[:], start=True, stop=True,
                                 skip_group_check=True)
            tmp_t = wp.tile([128, 2], F32, tag="tmp")
            nc.scalar.activation(tmp_t[:], ss_ps[:, 0:2], AF.Sqrt,
                                 scale=1.0 / HID, bias=eps_t[:])
            inv_t = wp.tile([128, 2], F32, tag="inv")
            nc.vector.reciprocal(inv_t[:], tmp_t[:])

            for blk in range(6):
                p, tb = blk % P, blk // P
                o_ps = psr.tile([128, HID], F32, tag="rzb", name=f"o_{blk}")
                for ci in range(4):
                    nc.tensor.matmul(o_ps[:],
                                     X_t[:, p, ci, tb * 128:(tb + 1) * 128],
                                     WT_t[:, ci, :],
                                     start=(ci == 0), stop=(ci == 3))
                o_sb = wp.tile([128, HID], F32, tag="osb")
                nc.vector.tensor_scalar_mul(o_sb[:], o_ps[:],
                                            inv_t[:, tb:tb + 1])
                nc.gpsimd.dma_start(out=out_d[tb * 128:(tb + 1) * 128, p, :],
                                    in_=o_sb[:])

    nc.compile()
    return nc


def _get_program():
    if "nc" not in _prog_cache:
        _prog_cache["nc"] = _build_program()
    return _prog_cache["nc"]


def _prepare_in_maps(q, k, v, attn_bias, key_padding_mask, outcell_index,
                     local_attention_weight, expand_mask, out_proj_weight,
                     attn_ln_weight):
    q = np.asarray(q, dtype=np.float32)
    k = np.asarray(k, dtype=np.float32)
    v = np.asarray(v, dtype=np.float32)
    attn_bias = np.asarray(attn_bias, dtype=np.float32)
    kpm = np.asarray(key_padding_mask)
    idx = np.asarray(outcell_index).astype(np.int64)
    law = np.asarray(local_attention_weight, dtype=np.float32)
    emask = np.asarray(expand_mask)
    W = np.asarray(out_proj_weight, dtype=np.float32)
    lnw = np.asarray(attn_ln_weight, dtype=np.float32)

    BF = ml_dtypes.bfloat16
    NEG = -1e30  # not -inf: the identity-matmul would make 0*(-inf)=NaN

    WT = np.ascontiguousarray((W * lnw[None, :]).T)  # [hid, o], ln folded
    idt_np = np.stack([_wrap_idx(np.arange(tt * 128, (tt + 1) * 128,
                                            dtype=np.int16))
                       for tt in range(2)])
    ones_np = np.ones((D, 1), dtype=np.float32)
    onesc_np = np.full((2, D), 1.0 / CS, dtype=ml_dtypes.bfloat16)
    eye_np = np.eye(128, dtype=BF)
    eyef_np = np.eye(128, dtype=np.float32)

    in_maps = []
    for c in range(8):
        b, th = c // 2, c % 2
        tsl = slice(th * TQ, (th + 1) * TQ)

        qT = q[b, tsl].reshape(TQ, P, H, HD).transpose(2, 1, 3, 0).reshape(H, D, TQ)
        kTl = k[b].reshape(T, P, H, HD).transpose(2, 1, 3, 0).reshape(H, D, T)
        kT = np.concatenate([kTl, kTl[:, :, idx[b]]], axis=2)  # [H, D, 1024]
        qkT = np.concatenate([kT, qT], axis=2)                 # [H, D, S+TQ]
        vpk = v[b].reshape(T, P, H, HD).transpose(0, 2, 1, 3).reshape(T, H * D)

        bias_c = np.ascontiguousarray(attn_bias[b, :, tsl, :])  # [H, 256, S]
        kpmS = np.concatenate([kpm[b], emask[b]])               # [S]
        if kpmS.any():
            bias_c[:, :, kpmS] = NEG
        cut = law[b, tsl] <= CUTOFF                             # [256, S]
        if cut.any():
            bias_c[:, cut] = NEG

        in_maps.append(dict(
            qkT=np.ascontiguousarray(qkT).astype(BF),
            vpk=vpk.astype(BF),
            bias=bias_c.reshape(H, 2, 128, S).astype(BF),
            law=np.ascontiguousarray(
                (law[b, tsl] * CS).reshape(2, 128, S)).astype(BF),
            WT=WT.astype(BF),
            idv=_wrap_idx(idx[b].astype(np.int16)),
            idt=idt_np,
            onesc=onesc_np,
            ones96=ones_np,
            eye128=eye_np,
            eyef128=eyef_np,
        ))
    return in_maps


def kernel(**inputs):
    in_maps = _prepare_in_maps(**inputs)
    nc = _get_program()
    res = run_bass_kernel_spmd(nc, in_maps, list(range(8)))

    out = np.empty((B, T, P, HID), dtype=np.float32)
    for c in range(8):
        b, th = c // 2, c % 2
        out[b, th * TQ:(th + 1) * TQ] = res.results[c]["out"]
    return out


# revision 28
# speedup vs baseline: 1.0101x; 1.0101x over previous
"""MemEffEquivariantAttention TRN2 Bass kernel (v4).

Sharding: 8 cores = 4 batches x 2 query-token halves (fully data-parallel,
no collectives). Each core computes, for its (batch, 256-token half):
scores -> +bias(masked) -> exp (no max; range-safe) -> u = e*law/Z ->
attn = u @ vf -> equivariant LN -> out_proj.

v4 design:
  - q/k bf16, packed in one qkT tensor (1 DMA trigger per head)
  - bias bf16, identity-matmul lands it in PSUM under the scores
  - Z-normalization DEFERRED off the critical path: u0 = e * (law*2^-32)
    (one 2x-mode bf16 tensor_tensor); the per-head attn output [96,256]
    is normalized instead, via rz broadcast built from a tiny PE
    transpose of z + a 1-partition ones(2^32)-matmul
  - attn outputs land partition-native in X2[96,(h),256]; out_proj runs
    as 96 matmuls with 32-wide contraction against a per-p replicated
    WT (heads 0-7 mid-pipeline, heads 8-15 in the tail) -- no X-stash
    DMA triggers at all
  - per-head pipeline (iteration h): dma(h+2) | scores(h) | exp(h) |
    u0(h) | gather(h) | ztr(h-1) | attn(h-2) | recip+bcast(h-1) |
    atnorm+sq(h-3) | out_proj passA block
"""
import sys
sys.path.insert(0, "/opt/trn_rl_repo")

import numpy as np
import ml_dtypes

import concourse.bacc as bacc
import concourse.tile as tile
from concourse import mybir
from concourse.bass_utils import run_bass_kernel_spmd

F32 = mybir.dt.float32
BF16 = mybir.dt.bfloat16
I16 = mybir.dt.int16
AF = mybir.ActivationFunctionType
ALU = mybir.AluOpType

B, T, P, HID = 4, 512, 3, 512
HD, H = 32, 16
EXP, S = 512, 1024
TQ = 256            # query tokens per core
EPS = 1e-3
CUTOFF = 1e-5
D = P * HD          # 96, per-head feature dim
CS = 2.0 ** -32     # u pre-scale; folded back via ones(2^32) bcast
LAG = 2             # attn(h) emitted at iteration h+LAG

_prog_cache = {}


def _wrap_idx(idx):
    # gpsimd wrapped layout, replicated to all 8 gpsimd cores:
    # idxs[p, s] = idx[s*16 + (p % 16)]
    n = len(idx)
    w = idx.reshape(n // 16, 16).T.astype(np.int16)
    return np.ascontiguousarray(np.tile(w, (8, 1)))


def _build_program():
    nc = bacc.Bacc("TRN2", target_bir_lowering=False, debug=False)

    qkT_d = nc.dram_tensor("qkT", [H, D, S + TQ], BF16, kind="ExternalInput").ap()
    vpk_d = nc.dram_tensor("vpk", [T, H * D], BF16, kind="ExternalInput").ap()
    bias_d = nc.dram_tensor("bias", [H, 2, 128, S], BF16, kind="ExternalInput").ap()
    law_d = nc.dram_tensor("law", [2, 128, S], BF16, kind="ExternalInput").ap()
    WT_d = nc.dram_tensor("WT", [HID, HID], BF16, kind="ExternalInput").ap()
    idv_d = nc.dram_tensor("idv", [128, 32], I16, kind="ExternalInput").ap()
    idt_d = nc.dram_tensor("idt", [2, 128, 8], I16, kind="ExternalInput").ap()
    onesc_d = nc.dram_tensor("onesc", [2, D], BF16, kind="ExternalInput").ap()
    ones_d = nc.dram_tensor("ones96", [D, 1], F32, kind="ExternalInput").ap()
    eye_d = nc.dram_tensor("eye128", [128, 128], BF16, kind="ExternalInput").ap()
    eyef_d = nc.dram_tensor("eyef128", [128, 128], F32, kind="ExternalInput").ap()
    out_d = nc.dram_tensor("out", [TQ, P, HID], F32, kind="ExternalOutput").ap()


    with tile.TileContext(nc) as tc:
        with tc.tile_pool(name="const", bufs=1) as cp, \
             tc.tile_pool(name="work", bufs=3) as wp, \
             tc.tile_pool(name="kq", bufs=3) as kq, \
             tc.tile_pool(name="ug", bufs=6) as ug, \
             tc.tile_pool(name="uT", bufs=6) as up, \
             tc.tile_pool(name="ep", bufs=3) as ep, \
             tc.tile_pool(name="biasp", bufs=3) as bp, \
             tc.tile_pool(name="psw", bufs=2, space="PSUM") as psw, \
             tc.tile_pool(name="psa", bufs=2, space="PSUM") as psa, \
             tc.tile_pool(name="psr", bufs=2, space="PSUM") as psr:

            # ---- constants / preload ----
            v_t = cp.tile([128, 4, H * D], BF16, tag="v")
            vg_t = cp.tile([128, 4, H * D], BF16, tag="vg")
            law_t = cp.tile([128, 2, S], BF16, tag="law")
            WT_t = cp.tile([128, 4, HID], BF16, tag="WT")
            idv_t = cp.tile([128, 32], I16, tag="idv")
            idt_t = cp.tile([128, 2, 8], I16, tag="idt")
            onesc_t = cp.tile([2, D], BF16, tag="onesc")
            ones_t = cp.tile([D, 1], F32, tag="ones")
            eye_t = cp.tile([128, 128], BF16, tag="eye")
            eyef_t = cp.tile([128, 128], F32, tag="eyef")
            X_t = cp.tile([128, P, 4, TQ], BF16, tag="X")
            eps_t = cp.tile([128, 1], F32, tag="eps")
            sqacc_t = cp.tile([D, TQ], F32, tag="sqacc")
            nc.vector.memset(eps_t[:], EPS)

            nc.sync.dma_start(out=eye_t[:], in_=eye_d)

            def emit_preload2():
                # everything not needed by scores(0)/exp(0): after the
                # first heads' input DMAs so they don't delay the start
                nc.sync.dma_start(out=idt_t[:],
                                  in_=idt_d.rearrange("r p c -> p r c"))
                nc.sync.dma_start(out=law_t[:],
                                  in_=law_d.rearrange("r p s -> p r s"))
                nc.sync.dma_start(out=idv_t[:], in_=idv_d)
                nc.sync.dma_start(out=onesc_t[:], in_=onesc_d)
                nc.sync.dma_start(out=eyef_t[:], in_=eyef_d)
                nc.sync.dma_start(out=v_t[:],
                                  in_=vpk_d.rearrange("(c p) d -> p c d", p=128))
                nc.sync.dma_start(out=WT_t[:],
                                  in_=WT_d.rearrange("(c p) o -> p c o", p=128))
                nc.sync.dma_start(out=ones_t[:], in_=ones_d)

            def emit_deferred_preload():
                # vg build on gpsimd after the first head's uT gathers
                nc.gpsimd.dma_gather(vg_t[:], vpk_d, idv_t[:],
                                     num_idxs=EXP, num_idxs_reg=EXP,
                                     elem_size=H * D)

            # per-head state carried between pipeline stages
            kq_t = {}
            bias_t = {}
            uT_tiles = {}
            at_tiles = {}
            at_tiles_sb = {}
            z_tiles = {}
            zrow_ps = {}
            rzrow_t = {}
            rzb_sb = {}

            def emit_dma(h):
                kq_t[h] = kq.tile([D, S + TQ], BF16, tag="kq", name=f"kq_{h}")
                bias_t[h] = bp.tile([128, 2, S], BF16, tag="bias",
                                    name=f"bias_{h}")
                nc.sync.dma_start(out=kq_t[h][:], in_=qkT_d[h])
                nc.sync.dma_start(out=bias_t[h][:],
                                  in_=bias_d[h].rearrange("r p s -> p r s"))

            def emit_scores(h):
                # PE: bias (identity) + scores into PSUM. tt-major order so
                # exp(tt0) overlaps the tt1 matmuls and the psw bank for
                # tt0 frees one exp earlier (breaks scores<->exp ping-pong)
                w_tiles = [psw.tile([128, S], F32, tag="w",
                                    name=f"w_{h}_{tt}") for tt in range(2)]
                for tt in range(2):
                    for half in range(2):
                        hs = slice(half * 512, (half + 1) * 512)
                        nc.tensor.matmul(w_tiles[tt][:, hs], eye_t[:],
                                         bias_t[h][:, tt, hs],
                                         start=True, stop=False,
                                         skip_group_check=True)
                    for half in range(2):
                        hs = slice(half * 512, (half + 1) * 512)
                        nc.tensor.matmul(w_tiles[tt][:, hs],
                                         kq_t[h][:, S + tt * 128:S + (tt + 1) * 128],
                                         kq_t[h][:, hs],
                                         start=False, stop=True,
                                         skip_group_check=True)
                return w_tiles

            def emit_exp_u(h, w_tiles):
                # per-tt chain: exp -> u0 -> rz stash -> HALF-gather, so the
                # tt0 gather DMA flies while tt1 is still in exp/DVE
                e_t = ep.tile([128, 2, S], BF16, tag="e", name=f"e_{h}")
                z_t = wp.tile([128, 2], F32, tag="z", name=f"z_{h}")
                rz_t = wp.tile([128, 2], F32, tag="z", name=f"rz_{h}")
                u1_t = ug.tile([128, 2, S + 128], BF16, tag="u1",
                               name=f"u1_{h}")

                uT_t = up.tile([128, 2, 9, 128], BF16, tag="uT",
                               name=f"uT_{h}")
                for tt in range(2):
                    nc.scalar.activation(e_t[:, tt, :], w_tiles[tt][:], AF.Exp,
                                         accum_out=z_t[:, tt:tt + 1])
                    nc.vector.tensor_mul(u1_t[:, tt, 0:S], e_t[:, tt, :],
                                         law_t[:, tt, :])
                    nc.vector.reciprocal(rz_t[:, tt:tt + 1],
                                         z_t[:, tt:tt + 1])
                    nc.vector.tensor_copy(u1_t[:, tt, S:S + 1],
                                          rz_t[:, tt:tt + 1])
                    nc.gpsimd.dma_gather(uT_t[:, tt, :, :],
                                         u1_t[:], idt_t[:, tt, :],
                                         num_idxs=128, num_idxs_reg=128,
                                         elem_size=S + 128, transpose=True,
                                         sbuf_tokens_per_rank=128,
                                         sbuf_free_dim_per_rank=2 * (S + 128))
                uT_tiles[h] = uT_t

            def emit_bcast(h):
                # rzb[0:96, t] = 2^32 * rz[t] broadcast down 96 partitions
                # (rz row came in via the gather: uT[0, 8, :]), then copied
                # to SBUF so the PSUM bank frees this iteration
                zr = psr.tile([128, HID], F32, tag="rzb", name=f"zr_{h}")
                for tt in range(2):
                    nc.tensor.matmul(zr[0:D, tt * 128:(tt + 1) * 128],
                                     onesc_t[0:1, :],
                                     uT_tiles[h][0:1, tt, 8, :],
                                     start=True, stop=True,
                                     skip_group_check=True)
                rzc = wp.tile([D, TQ], F32, tag="rzc", name=f"rzc_{h}")
                nc.vector.tensor_copy(rzc[:], zr[0:D, 0:256])
                rzb_sb[h] = rzc

            def emit_attn(h):
                uT_t = uT_tiles[h]
                at_ps = psa.tile([D, TQ], F32, tag="attn", name=f"at_{h}")
                for tt in range(2):
                    ts = slice(tt * 128, (tt + 1) * 128)
                    for sc in range(8):
                        vsrc = v_t if sc < 4 else vg_t
                        nc.tensor.matmul(at_ps[:, ts],
                                         vsrc[:, sc % 4, h * D:(h + 1) * D],
                                         uT_t[:, tt, sc, :],
                                         start=(sc == 0), stop=(sc == 7))
                at_tiles[h] = at_ps

            def emit_atnorm(h):
                # at_norm = at_ps * rzb -> at_sb; sq accumulate
                at_ps = at_tiles.pop(h)
                uT_tiles.pop(h)
                rzc = rzb_sb.pop(h)
                at_sb = wp.tile([D, TQ], BF16, tag="atsb", name=f"atsb_{h}")
                nc.vector.tensor_mul(at_sb[:], at_ps[:], rzc[:])
                at_tiles_sb[h] = at_sb
                if h == 0:
                    nc.vector.tensor_mul(sqacc_t[:], at_sb[:], at_sb[:])
                else:
                    sq_t = wp.tile([D, TQ], BF16, tag="sq")
                    nc.vector.tensor_mul(sq_t[:], at_sb[:], at_sb[:])
                    nc.vector.tensor_add(sqacc_t[:], sqacc_t[:], sq_t[:])

            def emit_stash(j):
                # X[(j%4)*32+hd, p, j//4, t] <- at_sb[p*32+hd, t]; deferred
                # one head so the triggers' waits are already satisfied
                at_sb = at_tiles_sb.pop(j)
                for p in range(P):
                    eng = nc.scalar if p == 2 else nc.sync
                    eng.dma_start(
                        out=X_t[(j % 4) * 32:(j % 4 + 1) * 32, p, j // 4, :],
                        in_=at_sb[p * 32:(p + 1) * 32, :])

            # ---- main loop ----
            emit_dma(0)
            emit_dma(1)
            emit_preload2()
            for h in range(H):
                if h + 2 < H:
                    emit_dma(h + 2)
                w_tiles = emit_scores(h)
                emit_exp_u(h, w_tiles)
                if h == 0:
                    emit_deferred_preload()
                if h >= LAG:
                    emit_attn(h - LAG)
                if h >= 1:
                    emit_bcast(h - 1)
                if h >= LAG + 1:
                    emit_atnorm(h - LAG - 1)
                if h >= LAG + 2:
                    emit_stash(h - LAG - 2)

            # drain
            emit_bcast(H - 1)
            for h in range(H - LAG, H):
                emit_attn(h)
            emit_stash(H - LAG - 2)
            for h in range(H - LAG - 1, H):
                emit_atnorm(h)
                emit_stash(h)

            # ---- inv = 1/sqrt(mean+eps), out_proj pass B, store ----
            ss_ps = psr.tile([128, HID], F32, tag="rzb", name="ss")
            for tb in range(2):
                nc.tensor.matmul(ss_ps[:, tb:tb + 1],
                                 sqacc_t[:, tb * 128:(tb + 1) * 128],
                                 ones_t[:], start=True, stop=True,
                                 skip_group_check=True)
            tmp_t = wp.tile([128, 2], F32, tag="tmp")
            nc.scalar.activation(tmp_t[:], ss_ps[:, 0:2], AF.Sqrt,
                                 scale=1.0 / HID, bias=eps_t[:])
            inv_t = wp.tile([128, 2], F32, tag="inv")
            nc.vector.reciprocal(inv_t[:], tmp_t[:])

            for blk in range(6):
                p, tb = blk % P, blk // P
                o_ps = psr.tile([128, HID], F32, tag="rzb", name=f"o_{blk}")
                for ci in range(4):
                    nc.tensor.matmul(o_ps[:],
                                     X_t[:, p, ci, tb * 128:(tb + 1) * 128],
                                     WT_t[:, ci, :],
                                     start=(ci == 0), stop=(ci == 3))
                o_sb = wp.tile([128, HID], F32, tag="osb")
                nc.vector.tensor_scalar_mul(o_sb[:], o_ps[:],
                                            inv_t[:, tb:tb + 1])
                nc.gpsimd.dma_start(out=out_d[tb * 128:(tb + 1) * 128, p, :],
                                    in_=o_sb[:])

    nc.compile()
    return nc


def _get_program():
    if "nc" not in _prog_cache:
        _prog_cache["nc"] = _build_program()
    return _prog_cache["nc"]


def _prepare_in_maps(q, k, v, attn_bias, key_padding_mask, outcell_index,
                     local_attention_weight, expand_mask, out_proj_weight,
                     attn_ln_weight):
    q = np.asarray(q, dtype=np.float32)
    k = np.asarray(k, dtype=np.float32)
    v = np.asarray(v, dtype=np.float32)
    attn_bias = np.asarray(attn_bias, dtype=np.float32)
    kpm = np.asarray(key_padding_mask)
    idx = np.asarray(outcell_index).astype(np.int64)
    law = np.asarray(local_attention_weight, dtype=np.float32)
    emask = np.asarray(expand_mask)
    W = np.asarray(out_proj_weight, dtype=np.float32)
    lnw = np.asarray(attn_ln_weight, dtype=np.float32)

    BF = ml_dtypes.bfloat16
    NEG = -1e30  # not -inf: the identity-matmul would make 0*(-inf)=NaN

    WT = np.ascontiguousarray((W * lnw[None, :]).T)  # [hid, o], ln folded
    idt_np = np.stack([_wrap_idx(np.arange(tt * 128, (tt + 1) * 128,
                                            dtype=np.int16))
                       for tt in range(2)])
    ones_np = np.ones((D, 1), dtype=np.float32)
    onesc_np = np.full((2, D), 1.0 / CS, dtype=ml_dtypes.bfloat16)
    eye_np = np.eye(128, dtype=BF)
    eyef_np = np.eye(128, dtype=np.float32)

    in_maps = []
    for c in range(8):
        b, th = c // 2, c % 2
        tsl = slice(th * TQ, (th + 1) * TQ)

        qT = q[b, tsl].reshape(TQ, P, H, HD).transpose(2, 1, 3, 0).reshape(H, D, TQ)
        kTl = k[b].reshape(T, P, H, HD).transpose(2, 1, 3, 0).reshape(H, D, T)
        kT = np.concatenate([kTl, kTl[:, :, idx[b]]], axis=2)  # [H, D, 1024]
        qkT = np.concatenate([kT, qT], axis=2)                 # [H, D, S+TQ]
        vpk = v[b].reshape(T, P, H, HD).transpose(0, 2, 1, 3).reshape(T, H * D)

        bias_c = np.ascontiguousarray(attn_bias[b, :, tsl, :])  # [H, 256, S]
        kpmS = np.concatenate([kpm[b], emask[b]])               # [S]
        if kpmS.any():
            bias_c[:, :, kpmS] = NEG
        cut = law[b, tsl] <= CUTOFF                             # [256, S]
        if cut.any():
            bias_c[:, cut] = NEG

        in_maps.append(dict(
            qkT=np.ascontiguousarray(qkT).astype(BF),
            vpk=vpk.astype(BF),
            bias=bias_c.reshape(H, 2, 128, S).astype(BF),
            law=np.ascontiguousarray(
                (law[b, tsl] * CS).reshape(2, 128, S)).astype(BF),
            WT=WT.astype(BF),
            idv=_wrap_idx(idx[b].astype(np.int16)),
            idt=idt_np,
            onesc=onesc_np,
            ones96=ones_np,
            eye128=eye_np,
            eyef128=eyef_np,
        ))
    return in_maps


def kernel(**inputs):
    in_maps = _prepare_in_maps(**inputs)
    nc = _get_program()
    res = run_bass_kernel_spmd(nc, in_maps, list(range(8)))

    out = np.empty((B, T, P, HID), dtype=np.float32)
    for c in range(8):
        b, th = c // 2, c % 2
        out[b, th * TQ:(th + 1) * TQ] = res.results[c]["out"]
    return out


# revision 29
# speedup vs baseline: 1.0222x; 1.0120x over previous
"""MemEffEquivariantAttention TRN2 Bass kernel (v4).

Sharding: 8 cores = 4 batches x 2 query-token halves (fully data-parallel,
no collectives). Each core computes, for its (batch, 256-token half):
scores -> +bias(masked) -> exp (no max; range-safe) -> u = e*law/Z ->
attn = u @ vf -> equivariant LN -> out_proj.

v4 design:
  - q/k bf16, packed in one qkT tensor (1 DMA trigger per head)
  - bias bf16, identity-matmul lands it in PSUM under the scores
  - Z-normalization DEFERRED off the critical path: u0 = e * (law*2^-32)
    (one 2x-mode bf16 tensor_tensor); the per-head attn output [96,256]
    is normalized instead, via rz broadcast built from a tiny PE
    transpose of z + a 1-partition ones(2^32)-matmul
  - attn outputs land partition-native in X2[96,(h),256]; out_proj runs
    as 96 matmuls with 32-wide contraction against a per-p replicated
    WT (heads 0-7 mid-pipeline, heads 8-15 in the tail) -- no X-stash
    DMA triggers at all
  - per-head pipeline (iteration h): dma(h+2) | scores(h) | exp(h) |
    u0(h) | gather(h) | ztr(h-1) | attn(h-2) | recip+bcast(h-1) |
    atnorm+sq(h-3) | out_proj passA block
"""
import sys
sys.path.insert(0, "/opt/trn_rl_repo")

import numpy as np
import ml_dtypes

import concourse.bacc as bacc
import concourse.tile as tile
from concourse import mybir
from concourse.bass_utils import run_bass_kernel_spmd

F32 = mybir.dt.float32
BF16 = mybir.dt.bfloat16
I16 = mybir.dt.int16
AF = mybir.ActivationFunctionType
ALU = mybir.AluOpType

B, T, P, HID = 4, 512, 3, 512
HD, H = 32, 16
EXP, S = 512, 1024
TQ = 256            # query tokens per core
EPS = 1e-3
CUTOFF = 1e-5
D = P * HD          # 96, per-head feature dim
CS = 2.0 ** -32     # u pre-scale; folded back via ones(2^32) bcast
LAG = 2             # attn(h) emitted at iteration h+LAG

_prog_cache = {}


def _wrap_idx(idx):
    # gpsimd wrapped layout, replicated to all 8 gpsimd cores:
    # idxs[p, s] = idx[s*16 + (p % 16)]
    n = len(idx)
    w = idx.reshape(n // 16, 16).T.astype(np.int16)
    return np.ascontiguousarray(np.tile(w, (8, 1)))


def _build_program():
    nc = bacc.Bacc("TRN2", target_bir_lowering=False, debug=False)

    qkT_d = nc.dram_tensor("qkT", [H, D, S + TQ], BF16, kind="ExternalInput").ap()
    vpk_d = nc.dram_tensor("vpk", [T, H * D], BF16, kind="ExternalInput").ap()
    bias_d = nc.dram_tensor("bias", [H, 2, 128, S], BF16, kind="ExternalInput").ap()
    law_d = nc.dram_tensor("law", [2, 128, S], BF16, kind="ExternalInput").ap()
    WT_d = nc.dram_tensor("WT", [HID, HID], BF16, kind="ExternalInput").ap()
    idv_d = nc.dram_tensor("idv", [128, 32], I16, kind="ExternalInput").ap()
    idt_d = nc.dram_tensor("idt", [2, 128, 8], I16, kind="ExternalInput").ap()
    onesc_d = nc.dram_tensor("onesc", [2, D], BF16, kind="ExternalInput").ap()
    ones_d = nc.dram_tensor("ones96", [D, 1], F32, kind="ExternalInput").ap()
    eye_d = nc.dram_tensor("eye128", [128, 128], BF16, kind="ExternalInput").ap()
    eyef_d = nc.dram_tensor("eyef128", [128, 128], F32, kind="ExternalInput").ap()
    out_d = nc.dram_tensor("out", [TQ, P, HID], F32, kind="ExternalOutput").ap()


    with tile.TileContext(nc) as tc:
        with tc.tile_pool(name="const", bufs=1) as cp, \
             tc.tile_pool(name="work", bufs=3) as wp, \
             tc.tile_pool(name="kq", bufs=3) as kq, \
             tc.tile_pool(name="ug", bufs=6) as ug, \
             tc.tile_pool(name="uT", bufs=6) as up, \
             tc.tile_pool(name="ep", bufs=3) as ep, \
             tc.tile_pool(name="biasp", bufs=3) as bp, \
             tc.tile_pool(name="psw", bufs=2, space="PSUM") as psw, \
             tc.tile_pool(name="psa", bufs=2, space="PSUM") as psa, \
             tc.tile_pool(name="psr", bufs=2, space="PSUM") as psr:

            # ---- constants / preload ----
            v_t = cp.tile([128, 4, H * D], BF16, tag="v")
            vg_t = cp.tile([128, 4, H * D], BF16, tag="vg")
            law_t = cp.tile([128, 2, S], BF16, tag="law")
            WT_t = cp.tile([128, 4, HID], BF16, tag="WT")
            idv_t = cp.tile([128, 32], I16, tag="idv")
            idt_t = cp.tile([128, 2, 8], I16, tag="idt")
            onesc_t = cp.tile([2, D], BF16, tag="onesc")
            ones_t = cp.tile([D, 1], F32, tag="ones")
            eye_t = cp.tile([128, 128], BF16, tag="eye")
            eyef_t = cp.tile([128, 128], F32, tag="eyef")
            X_t = cp.tile([128, P, 4, TQ], BF16, tag="X")
            eps_t = cp.tile([128, 1], F32, tag="eps")
            sqacc_t = cp.tile([D, TQ], F32, tag="sqacc")
            nc.vector.memset(eps_t[:], EPS)

            nc.sync.dma_start(out=eye_t[:], in_=eye_d)

            def emit_preload2():
                # everything not needed by scores(0)/exp(0): after the
                # first heads' input DMAs so they don't delay the start
                nc.sync.dma_start(out=idt_t[:],
                                  in_=idt_d.rearrange("r p c -> p r c"))
                nc.sync.dma_start(out=law_t[:],
                                  in_=law_d.rearrange("r p s -> p r s"))
                nc.sync.dma_start(out=idv_t[:], in_=idv_d)
                nc.sync.dma_start(out=onesc_t[:], in_=onesc_d)
                nc.sync.dma_start(out=eyef_t[:], in_=eyef_d)

            def emit_deferred_preload():
                nc.gpsimd.dma_start(out=v_t[:],
                                    in_=vpk_d.rearrange("(c p) d -> p c d", p=128))
                nc.gpsimd.dma_gather(vg_t[:], vpk_d, idv_t[:],
                                     num_idxs=EXP, num_idxs_reg=EXP,
                                     elem_size=H * D)
                nc.gpsimd.dma_start(out=WT_t[:],
                                    in_=WT_d.rearrange("(c p) o -> p c o", p=128))
                nc.gpsimd.dma_start(out=ones_t[:], in_=ones_d)

            # per-head state carried between pipeline stages
            kq_t = {}
            bias_t = {}
            uT_tiles = {}
            at_tiles = {}
            at_tiles_sb = {}
            z_tiles = {}
            zrow_ps = {}
            rzrow_t = {}
            rzb_sb = {}

            def emit_dma(h):
                kq_t[h] = kq.tile([D, S + TQ], BF16, tag="kq", name=f"kq_{h}")
                bias_t[h] = bp.tile([128, 2, S], BF16, tag="bias",
                                    name=f"bias_{h}")
                nc.sync.dma_start(out=kq_t[h][:], in_=qkT_d[h])
                nc.sync.dma_start(out=bias_t[h][:],
                                  in_=bias_d[h].rearrange("r p s -> p r s"))

            def emit_scores(h):
                # PE: bias (identity) + scores into PSUM. tt-major order so
                # exp(tt0) overlaps the tt1 matmuls and the psw bank for
                # tt0 frees one exp earlier (breaks scores<->exp ping-pong)
                w_tiles = [psw.tile([128, S], F32, tag="w",
                                    name=f"w_{h}_{tt}") for tt in range(2)]
                for tt in range(2):
                    for half in range(2):
                        hs = slice(half * 512, (half + 1) * 512)
                        nc.tensor.matmul(w_tiles[tt][:, hs], eye_t[:],
                                         bias_t[h][:, tt, hs],
                                         start=True, stop=False,
                                         skip_group_check=True)
                    for half in range(2):
                        hs = slice(half * 512, (half + 1) * 512)
                        nc.tensor.matmul(w_tiles[tt][:, hs],
                                         kq_t[h][:, S + tt * 128:S + (tt + 1) * 128],
                                         kq_t[h][:, hs],
                                         start=False, stop=True,
                                         skip_group_check=True)
                return w_tiles

            def emit_exp_u(h, w_tiles):
                # per-tt chain: exp -> u0 -> rz stash -> HALF-gather, so the
                # tt0 gather DMA flies while tt1 is still in exp/DVE
                e_t = ep.tile([128, 2, S], BF16, tag="e", name=f"e_{h}")
                z_t = wp.tile([128, 2], F32, tag="z", name=f"z_{h}")
                rz_t = wp.tile([128, 2], F32, tag="z", name=f"rz_{h}")
                u1_t = ug.tile([128, 2, S + 128], BF16, tag="u1",
                               name=f"u1_{h}")

                uT_t = up.tile([128, 2, 9, 128], BF16, tag="uT",
                               name=f"uT_{h}")
                for tt in range(2):
                    nc.scalar.activation(e_t[:, tt, :], w_tiles[tt][:], AF.Exp,
                                         accum_out=z_t[:, tt:tt + 1])
                    nc.vector.tensor_mul(u1_t[:, tt, 0:S], e_t[:, tt, :],
                                         law_t[:, tt, :])
                    nc.vector.reciprocal(rz_t[:, tt:tt + 1],
                                         z_t[:, tt:tt + 1])
                    nc.vector.tensor_copy(u1_t[:, tt, S:S + 1],
                                          rz_t[:, tt:tt + 1])
                    nc.gpsimd.dma_gather(uT_t[:, tt, :, :],
                                         u1_t[:], idt_t[:, tt, :],
                                         num_idxs=128, num_idxs_reg=128,
                                         elem_size=S + 128, transpose=True,
                                         sbuf_tokens_per_rank=128,
                                         sbuf_free_dim_per_rank=2 * (S + 128))
                uT_tiles[h] = uT_t

            def emit_bcast(h):
                # rzb[0:96, t] = 2^32 * rz[t] broadcast down 96 partitions
                # (rz row came in via the gather: uT[0, 8, :]), then copied
                # to SBUF so the PSUM bank frees this iteration
                zr = psr.tile([128, HID], F32, tag="rzb", name=f"zr_{h}")
                for tt in range(2):
                    nc.tensor.matmul(zr[0:D, tt * 128:(tt + 1) * 128],
                                     onesc_t[0:1, :],
                                     uT_tiles[h][0:1, tt, 8, :],
                                     start=True, stop=True,
                                     skip_group_check=True)
                rzc = wp.tile([D, TQ], F32, tag="rzc", name=f"rzc_{h}")
                nc.vector.tensor_copy(rzc[:], zr[0:D, 0:256])
                rzb_sb[h] = rzc

            def emit_attn(h):
                uT_t = uT_tiles[h]
                at_ps = psa.tile([D, TQ], F32, tag="attn", name=f"at_{h}")
                for tt in range(2):
                    ts = slice(tt * 128, (tt + 1) * 128)
                    for sc in range(8):
                        vsrc = v_t if sc < 4 else vg_t
                        nc.tensor.matmul(at_ps[:, ts],
                                         vsrc[:, sc % 4, h * D:(h + 1) * D],
                                         uT_t[:, tt, sc, :],
                                         start=(sc == 0), stop=(sc == 7))
                at_tiles[h] = at_ps

            def emit_atnorm(h):
                # at_norm = at_ps * rzb -> at_sb; sq accumulate
                at_ps = at_tiles.pop(h)
                uT_tiles.pop(h)
                rzc = rzb_sb.pop(h)
                at_sb = wp.tile([D, TQ], BF16, tag="atsb", name=f"atsb_{h}")
                nc.vector.tensor_mul(at_sb[:], at_ps[:], rzc[:])
                at_tiles_sb[h] = at_sb
                if h == 0:
                    nc.vector.tensor_mul(sqacc_t[:], at_sb[:], at_sb[:])
                else:
                    sq_t = wp.tile([D, TQ], BF16, tag="sq")
                    nc.vector.tensor_mul(sq_t[:], at_sb[:], at_sb[:])
                    nc.vector.tensor_add(sqacc_t[:], sqacc_t[:], sq_t[:])

            def emit_stash(j):
                # X[(j%4)*32+hd, p, j//4, t] <- at_sb[p*32+hd, t]; deferred
                # one head so the triggers' waits are already satisfied
                at_sb = at_tiles_sb.pop(j)
                for p in range(P):
                    eng = nc.scalar if p == 2 else nc.sync
                    eng.dma_start(
                        out=X_t[(j % 4) * 32:(j % 4 + 1) * 32, p, j // 4, :],
                        in_=at_sb[p * 32:(p + 1) * 32, :])

            # ---- main loop ----
            emit_dma(0)
            emit_dma(1)
            emit_preload2()
            for h in range(H):
                if h + 2 < H:
                    emit_dma(h + 2)
                w_tiles = emit_scores(h)
                if h == 0:
                    emit_deferred_preload()
                emit_exp_u(h, w_tiles)
                if h >= LAG:
                    emit_attn(h - LAG)
                if h >= 1:
                    emit_bcast(h - 1)
                if h >= LAG + 1:
                    emit_atnorm(h - LAG - 1)
                if h >= LAG + 2:
                    emit_stash(h - LAG - 2)

            # drain
            emit_bcast(H - 1)
            for h in range(H - LAG, H):
                emit_attn(h)
            for h in range(H - LAG - 1, H):
                emit_atnorm(h)
            for h in range(H - LAG - 2, H):
                emit_stash(h)

            # ---- inv = 1/sqrt(mean+eps), out_proj pass B, store ----
            ss_ps = psr.tile([128, HID], F32, tag="rzb", name="ss")
            for tb in range(2):
                nc.tensor.matmul(ss_ps[:, tb:tb + 1],
                                 sqacc_t[:, tb * 128:(tb + 1) * 128],
                                 ones_t[:], start=True, stop=True,
                                 skip_group_check=True)
            tmp_t = wp.tile([128, 2], F32, tag="tmp")
            nc.scalar.activation(tmp_t[:], ss_ps[:, 0:2], AF.Sqrt,
                                 scale=1.0 / HID, bias=eps_t[:])
            inv_t = wp.tile([128, 2], F32, tag="inv")
            nc.vector.reciprocal(inv_t[:], tmp_t[:])

            for blk in range(6):
                p, tb = blk % P, blk // P
                o_ps = psr.tile([128, HID], F32, tag="rzb", name=f"o_{blk}")
                for ci in range(4):
                    nc.tensor.matmul(o_ps[:],
                                     X_t[:, p, ci, tb * 128:(tb + 1) * 128],
                                     WT_t[:, ci, :],
                                     start=(ci == 0), stop=(ci == 3))
                o_sb = wp.tile([128, HID], F32, tag="osb")
                nc.vector.tensor_scalar_mul(o_sb[:], o_ps[:],
                                            inv_t[:, tb:tb + 1])
                nc.gpsimd.dma_start(out=out_d[tb * 128:(tb + 1) * 128, p, :],
                                    in_=o_sb[:])

    nc.compile()
    return nc


def _get_program():
    if "nc" not in _prog_cache:
        _prog_cache["nc"] = _build_program()
    return _prog_cache["nc"]


def _prepare_in_maps(q, k, v, attn_bias, key_padding_mask, outcell_index,
                     local_attention_weight, expand_mask, out_proj_weight,
                     attn_ln_weight):
    q = np.asarray(q, dtype=np.float32)
    k = np.asarray(k, dtype=np.float32)
    v = np.asarray(v, dtype=np.float32)
    attn_bias = np.asarray(attn_bias, dtype=np.float32)
    kpm = np.asarray(key_padding_mask)
    idx = np.asarray(outcell_index).astype(np.int64)
    law = np.asarray(local_attention_weight, dtype=np.float32)
    emask = np.asarray(expand_mask)
    W = np.asarray(out_proj_weight, dtype=np.float32)
    lnw = np.asarray(attn_ln_weight, dtype=np.float32)

    BF = ml_dtypes.bfloat16
    NEG = -1e30  # not -inf: the identity-matmul would make 0*(-inf)=NaN

    WT = np.ascontiguousarray((W * lnw[None, :]).T)  # [hid, o], ln folded
    idt_np = np.stack([_wrap_idx(np.arange(tt * 128, (tt + 1) * 128,
                                            dtype=np.int16))
                       for tt in range(2)])
    ones_np = np.ones((D, 1), dtype=np.float32)
    onesc_np = np.full((2, D), 1.0 / CS, dtype=ml_dtypes.bfloat16)
    eye_np = np.eye(128, dtype=BF)
    eyef_np = np.eye(128, dtype=np.float32)

    in_maps = []
    for c in range(8):
        b, th = c // 2, c % 2
        tsl = slice(th * TQ, (th + 1) * TQ)

        qT = q[b, tsl].reshape(TQ, P, H, HD).transpose(2, 1, 3, 0).reshape(H, D, TQ)
        kTl = k[b].reshape(T, P, H, HD).transpose(2, 1, 3, 0).reshape(H, D, T)
        kT = np.concatenate([kTl, kTl[:, :, idx[b]]], axis=2)  # [H, D, 1024]
        qkT = np.concatenate([kT, qT], axis=2)                 # [H, D, S+TQ]
        vpk = v[b].reshape(T, P, H, HD).transpose(0, 2, 1, 3).reshape(T, H * D)

        bias_c = np.ascontiguousarray(attn_bias[b, :, tsl, :])  # [H, 256, S]
        kpmS = np.concatenate([kpm[b], emask[b]])               # [S]
        if kpmS.any():
            bias_c[:, :, kpmS] = NEG
        cut = law[b, tsl] <= CUTOFF                             # [256, S]
        if cut.any():
            bias_c[:, cut] = NEG

        in_maps.append(dict(
            qkT=np.ascontiguousarray(qkT).astype(BF),
            vpk=vpk.astype(BF),
            bias=bias_c.reshape(H, 2, 128, S).astype(BF),
            law=np.ascontiguousarray(
                (law[b, tsl] * CS).reshape(2, 128, S)).astype(BF),
            WT=WT.astype(BF),
            idv=_wrap_idx(idx[b].astype(np.int16)),
            idt=idt_np,
            onesc=onesc_np,
            ones96=ones_np,
            eye128=eye_np,
            eyef128=eyef_np,
        ))
    return in_maps


def kernel(**inputs):
    in_maps = _prepare_in_maps(**inputs)
    nc = _get_program()
    res = run_bass_kernel_spmd(nc, in_maps, list(range(8)))

    out = np.empty((B, T, P, HID), dtype=np.float32)
    for c in range(8):
        b, th = c // 2, c % 2
        out[b, th * TQ:(th + 1) * TQ] = res.results[c]["out"]
    return out
